# revision 21
# baseline (speedup 1.0000x reference)
"""Multi-head causal attention (B=4, T=2048, D=1024, H=16, hd=64) on 8 trn2 cores.

Sharding: core = (batch, head-half). Each core runs the full attention for its
batch element over 8 heads and produces the partial output projection against
its 512 rows of Wo. Host sums the two partials per batch and adds bo. No
device-to-device communication.

Per-core pipeline (Tile framework), v2:
  1. x -> xT via PE transpose instr (f32, 2 cycles/row); Q/K projections in
     f32r (full PE rate at K=128) writing fp16 qT/kT; V projected directly in
     [t, e] layout (stationary = xT chunk, moving = Wv for all 8 heads), so no
     separate V transpose.
  2. Per (head, 128-row q-block): scores = qT.T @ kT chunks in fp16 (1 c/r);
     causal masking + row-max of the diagonal 128-block via one
     tensor_mask_reduce (per-row exact mask, accum=max); shift C additionally
     samples the first <=256 columns. C bounds the true row max to within
     ~tail nats w.h.p.; margin -56 keeps exp finite and the denominator
     >= e^-56; exp + row-sum fused on ScalarE (accum_out).
  3. p (bf16) normalized by 1/sum on GpSimd (idle engine), transposed by the
     DMA xbar (sync engine issues).
  4. att @ V accumulated per 512-wide q-superblock (bf16 x bf16, fp32 PSUM).
  5. Output projection vs fp16 Wo shard; partial written to DRAM. x loads and
     out stores are issued from GpSimd to keep Scalar free for exp.
"""

import os
import sys
from contextlib import ExitStack

import numpy as np

if "/opt/trn_rl_repo" not in sys.path:
    sys.path.insert(0, "/opt/trn_rl_repo")

import concourse.bass as bass
import concourse.bacc as bacc
import concourse.mybir as mybir
import concourse.tile as tile
from concourse.masks import make_causal_mask, make_identity

F32 = mybir.dt.float32
F32R = mybir.dt.float32r
BF16 = mybir.dt.bfloat16
FP16 = mybir.dt.float16

MARGIN = -56.0


def r32(ap):
    return ap.bitcast(F32R)


USE_GPS_NORM = bool(int(os.environ.get("K_GPSN", "1")))
USE_XTR = bool(int(os.environ.get("K_XTR", "1")))
USE_FP16 = bool(int(os.environ.get("K_FP16", "1")))


def build_program(T=2048, D=1024, HL=8, HD=64):
    """One SPMD program: full attention for one batch element and HL local
    heads, plus the partial output projection (HL*HD rows of Wo).

    Fully fused pipeline over 512-row t-blocks: QKV(tb) -> shift C(tb) ->
    attention(qsb=tb, all heads) -> partial out-projection(tb). Keeps PE busy
    with projection matmuls while ScalarE runs exp for the previous block.
    """
    SCALE = float(HD) ** 0.5
    QKDT = FP16 if USE_FP16 else F32R
    FTDT = FP16 if USE_FP16 else BF16
    DC = D // 128          # contraction chunks for projections
    TB = T // 512          # 512-wide t blocks
    NSC = T // 128         # 128-wide s chunks
    HP = HL // 2           # head pairs
    HEC = (HL * HD) // 128 # feature contraction chunks for out proj
    DH = D // 512          # output column blocks

    nc = bacc.Bacc()

    x_d = nc.declare_dram_parameter("x", [T, D], F32, isOutput=False)
    wq_d = nc.declare_dram_parameter("wq", [HL, D, HD], F32R, isOutput=False)
    wk_d = nc.declare_dram_parameter("wk", [HL, D, HD], F32R, isOutput=False)
    wv_d = nc.declare_dram_parameter("wv", [HL, D, HD], F32R, isOutput=False)
    bq_d = nc.declare_dram_parameter("bq", [HL, HD], F32, isOutput=False)
    bk_d = nc.declare_dram_parameter("bk", [HL, HD], F32, isOutput=False)
    bv_d = nc.declare_dram_parameter("bv", [HL, HD], F32, isOutput=False)
    wo_d = nc.declare_dram_parameter("wo", [HL * HD, D], F32, isOutput=False)
    out_d = nc.declare_dram_parameter("out", [T, D], F32, isOutput=True)

    with tile.TileContext(nc) as tc, ExitStack() as ctx:
        const = ctx.enter_context(tc.tile_pool(name="const", bufs=1))
        pers = ctx.enter_context(tc.tile_pool(name="pers", bufs=1))
        small = ctx.enter_context(tc.tile_pool(name="small", bufs=8))
        xpool = ctx.enter_context(tc.tile_pool(name="xpool", bufs=2))
        xtpool = ctx.enter_context(tc.tile_pool(name="xtpool", bufs=1))
        qtpool = ctx.enter_context(tc.tile_pool(name="qtpool", bufs=2))
        ftpool = ctx.enter_context(tc.tile_pool(name="ftpool", bufs=2))
        ppool = ctx.enter_context(tc.tile_pool(name="ppool", bufs=3))
        ptpool = ctx.enter_context(tc.tile_pool(name="ptpool", bufs=2))
        opool = ctx.enter_context(tc.tile_pool(name="opool", bufs=2))
        ps512 = ctx.enter_context(tc.tile_pool(name="ps512", bufs=2, space="PSUM"))
        pssc = ctx.enter_context(tc.tile_pool(name="pssc", bufs=2, space="PSUM"))
        psv = ctx.enter_context(tc.tile_pool(name="psv", bufs=1, space="PSUM"))
        psO = ctx.enter_context(tc.tile_pool(name="psO", bufs=1, space="PSUM"))

        # ---- constants ----
        id_f32 = const.tile([128, 128], F32)
        make_identity(nc, id_f32)

        tri = const.tile([128, 128], F32)
        make_causal_mask(nc, tri, mask_val=-1.0e30)

        bqkv_sb = const.tile([128, 2, HP], F32)  # (h2 e) stacked per head pair
        for i, b_d in enumerate((bq_d, bk_d)):
            nc.scalar.dma_start(
                out=bqkv_sb[:, i, :],
                in_=b_d[:].rearrange("(hp h2) e -> (h2 e) hp", h2=2),
            )
        # bv broadcast to [t=128, (hp h2 e)=512] via PE outer product
        ones_row = const.tile([1, 128], F32)
        nc.gpsimd.memset(ones_row, 1.0)
        bv_row = const.tile([1, HL * HD], F32)
        nc.scalar.dma_start(out=bv_row, in_=bv_d[:].rearrange("g e -> (g e)"))
        bv_bc = const.tile([128, HL * HD], F32)
        ps_bc = psO.tile([128, 512], F32, tag="op", name="bvbc")
        nc.tensor.matmul(ps_bc, ones_row, bv_row, start=True, stop=True)
        nc.vector.tensor_copy(bv_bc, ps_bc)

        # prefetch the first two x chunks ahead of the weight stream
        x_pre = []
        for tc4 in range(2):
            x_tc = xpool.tile([128, D], F32, tag="x", name=f"x_pre{tc4}")
            nc.scalar.dma_start(out=x_tc, in_=x_d[tc4 * 128 : (tc4 + 1) * 128, :])
            x_pre.append(x_tc)
        # weights: one batched DMA per projection chunk
        w_sb = []
        for wi in range(3):
            wt = pers.tile([128, DC, HP, 2, HD], F32R, tag=f"w{wi}", name=f"w{wi}")
            w_sb.append(wt)
        for dc in range(DC):
            for wi, w_d in enumerate((wq_d, wk_d, wv_d)):
                nc.scalar.dma_start(
                    out=w_sb[wi][:, dc].rearrange("p hp h2 e -> p (hp h2) e"),
                    in_=w_d[:, dc * 128 : (dc + 1) * 128, :].rearrange(
                        "g d e -> d g e"
                    ),
                )
        # Wo: load f32 per hec chunk via the x-sized staging tag, cast to fp16
        wo_hf = pers.tile([128, HEC, D], FTDT)
        for hec in range(HEC):
            wo_st = xpool.tile([128, D], F32, tag="x", name=f"wo_st{hec}")
            nc.scalar.dma_start(
                out=wo_st, in_=wo_d[hec * 128 : (hec + 1) * 128, :]
            )
            nc.vector.tensor_copy(wo_hf[:, hec], wo_st)

        # ---- persistent activations (split per t block for precise deps) ----
        kT_t = [pers.tile([128, HP, 512], QKDT, tag=f"kT{i}", name=f"kT{i}") for i in range(TB)]
        v_t = [pers.tile([128, 4, HL * HD], BF16, tag=f"v{i}", name=f"v{i}") for i in range(TB)]

        for tb in range(TB):
            # ======== QKV projections for this t block ========
            xT_blk = xtpool.tile([128, DC, 512], F32R, tag="xT")
            qT_t = qtpool.tile([128, HP, 512], QKDT, tag="qT")
            for tc4 in range(4):
                if tb == 0 and tc4 < 2:
                    x_tc = x_pre[tc4]
                else:
                    x_tc = xpool.tile([128, D], F32, tag="x")
                    row0 = tb * 512 + tc4 * 128
                    nc.scalar.dma_start(out=x_tc, in_=x_d[row0 : row0 + 128, :])
                for dcg in range(DC // 4):
                    ps = ps512.tile([128, 4, 128], F32, tag="mm")
                    for j in range(4):
                        dc = dcg * 4 + j
                        if USE_XTR:
                            nc.tensor.transpose(
                                ps[:, j], x_tc[:, dc * 128 : (dc + 1) * 128],
                                id_f32,
                            )
                        else:
                            nc.tensor.matmul(
                                ps[:, j], x_tc[:, dc * 128 : (dc + 1) * 128],
                                id_f32, start=True, stop=True,
                            )
                    nc.vector.tensor_copy(
                        xT_blk[:, dcg * 4 : dcg * 4 + 4,
                               tc4 * 128 : (tc4 + 1) * 128],
                        ps,
                    )
            for hp in range(HP):
                for wi in range(2):
                    ps = ps512.tile([128, 512], F32, tag="mm")
                    for dc in range(DC):
                        nc.tensor.matmul(
                            ps, w_sb[wi][:, dc, hp], xT_blk[:, dc],
                            start=(dc == 0), stop=(dc == DC - 1),
                        )
                    bias_ap = bqkv_sb[:, wi, hp : hp + 1]
                    if wi == 0:
                        # fold the sqrt(hd) score scale into Q so the exp
                        # activation can run with scale=1.0 (keeps the
                        # -FLT_MAX causal-mask fill finite through exp)
                        nc.vector.tensor_scalar(
                            out=qT_t[:, hp], in0=ps,
                            scalar1=bias_ap, scalar2=SCALE,
                            op0=mybir.AluOpType.add,
                            op1=mybir.AluOpType.mult,
                        )
                    else:
                        nc.vector.tensor_scalar_add(kT_t[tb][:, hp], ps, bias_ap)
            # V directly in [t, (hp h2 e)] layout: stationary = xT chunk
            for tc4 in range(4):
                ps = ps512.tile([128, 512], F32, tag="mm")
                for dc in range(DC):
                    nc.tensor.matmul(
                        ps,
                        xT_blk[:, dc, tc4 * 128 : (tc4 + 1) * 128],
                        w_sb[2][:, dc],
                        start=(dc == 0), stop=(dc == DC - 1),
                    )
                nc.vector.tensor_add(v_t[tb][:, tc4], ps, bv_bc)

            # ======== attention for q superblock qsb == tb ========
            featT_t = ftpool.tile([128, HEC, 512], FTDT, tag="ft")
            smax = (tb + 1) * 4
            for hp in range(HP):
                pT_pair = [
                    ptpool.tile([128, NSC, 512], BF16, tag="pT", name=f"pT{i}")
                    for i in range(2)
                ]
                for q4 in range(4):
                    qb = tb * 4 + q4
                    slen = (qb + 1) * 128
                    nch = (slen + 1023) // 1024
                    p2 = [ppool.tile([128, T], BF16, tag="p", name=f"p{i}")
                          for i in range(2)]
                    sums2 = [small.tile([128, 2], F32, tag="sums",
                                        name=f"sums{i}") for i in range(2)]
                    cneg2 = [None, None]
                    for c in range(nch):
                        lo = c * 1024
                        w = min(1024, slen - lo)
                        ps2 = [pssc.tile([128, 1024], F32, tag="sc",
                                         name=f"sc{i}") for i in range(2)]
                        # paired row-disjoint matmuls on PE
                        for m0 in range(0, w, 512):
                            mw = min(512, w - m0)
                            kt = kT_t[(lo + m0) // 512]
                            for h2 in range(2):
                                e0, e1 = h2 * HD, (h2 + 1) * HD
                                nc.tensor.matmul(
                                    ps2[h2][:, m0 : m0 + mw],
                                    qT_t[e0:e1, hp,
                                         q4 * 128 : (q4 + 1) * 128],
                                    kt[e0:e1, hp, :mw],
                                    start=True, stop=True,
                                )
                        for h2 in range(2):
                            is_diag = c == nch - 1
                            if is_diag:
                                # causal mask of the diagonal 128 block
                                nc.vector.tensor_add(
                                    ps2[h2][:, w - 128 : w],
                                    ps2[h2][:, w - 128 : w], tri,
                                )
                            # shift C from cheap samples: the (masked) diag
                            # block max (nch==1) and/or the first <=256
                            # columns; cm <= true row max always, and trails
                            # it by few enough nats w.h.p. that margin -56
                            # keeps exp finite and the denominator >= e^-56
                            if c == 0 and qb >= 1:
                                kw = min(256, qb * 128)
                                cm_s = small.tile([128, 1], F32, tag="cm",
                                                  name=f"cms{h2}")
                                nc.vector.reduce_max(
                                    cm_s, ps2[h2][:, :kw],
                                    axis=mybir.AxisListType.X,
                                )
                            elif c == 0:
                                cm_s = None
                            if is_diag and nch == 1:
                                cm_d = small.tile([128, 1], F32, tag="cm",
                                                  name=f"cmd{h2}")
                                nc.vector.reduce_max(
                                    cm_d, ps2[h2][:, w - 128 : w],
                                    axis=mybir.AxisListType.X,
                                )
                            if c == 0:
                                if nch == 1 and cm_s is not None:
                                    cm = small.tile([128, 1], F32, tag="cm",
                                                    name=f"cmx{h2}")
                                    nc.vector.tensor_tensor(
                                        out=cm, in0=cm_d, in1=cm_s,
                                        op=mybir.AluOpType.max,
                                    )
                                elif nch == 1:
                                    cm = cm_d
                                else:
                                    cm = cm_s
                                cneg_t = small.tile([128, 1], F32, tag="cneg",
                                                    name=f"cneg{h2}")
                                nc.vector.tensor_scalar(
                                    out=cneg_t, in0=cm,
                                    scalar1=-1.0, scalar2=MARGIN,
                                    op0=mybir.AluOpType.mult,
                                    op1=mybir.AluOpType.add,
                                )
                                cneg2[h2] = cneg_t
                            nc.scalar.activation(
                                out=p2[h2][:, lo : lo + w], in_=ps2[h2][:, :w],
                                func=mybir.ActivationFunctionType.Exp,
                                bias=cneg2[h2], scale=1.0,
                                accum_out=sums2[h2][:, c : c + 1],
                            )
                    for h2 in range(2):
                        if nch == 1:
                            ssum = sums2[h2][:, 0:1]
                        else:
                            ssum = small.tile([128, 1], F32, tag="ssum")
                            nc.vector.tensor_tensor(
                                out=ssum, in0=sums2[h2][:, 0:1],
                                in1=sums2[h2][:, 1:2], op=mybir.AluOpType.add,
                            )
                        rinv = small.tile([128, 1], F32, tag="rinv")
                        nc.vector.reciprocal(rinv, ssum)
                        norm_eng = nc.gpsimd if USE_GPS_NORM else nc.vector
                        norm_eng.tensor_scalar_mul(
                            p2[h2][:, :slen], p2[h2][:, :slen], rinv
                        )
                        nc.sync.dma_start_transpose(
                            out=pT_pair[h2][:, 0 : qb + 1,
                                            q4 * 128 : (q4 + 1) * 128],
                            in_=p2[h2][:, :slen],
                        )
                # att @ V: both heads interleaved -> col-groups 0-63 / 64-127
                psva = psv.tile([128, 512], F32, tag="attv")
                for sc in range(smax):
                    j = max(0, sc - tb * 4)
                    t_lo = j * 128
                    for h2 in range(2):
                        nc.tensor.matmul(
                            psva[h2 * HD : (h2 + 1) * HD, t_lo:],
                            v_t[sc // 4][:, sc % 4,
                                         hp * 128 + h2 * HD :
                                         hp * 128 + h2 * HD + HD],
                            pT_pair[h2][:, sc, t_lo:],
                            start=(sc == 0), stop=(sc == smax - 1),
                            skip_group_check=True,
                        )
                nc.vector.tensor_copy(featT_t[:, hp], psva)

            # ======== partial out-projection for this t block ========
            ftv = featT_t.rearrange("p hec (t4 t) -> p hec t4 t", t=128)
            for t4 in range(4):
                for dh in range(DH):
                    ps = psO.tile([128, 512], F32, tag="op")
                    for hec in range(HEC):
                        nc.tensor.matmul(
                            ps, ftv[:, hec, t4],
                            wo_hf[:, hec, dh * 512 : (dh + 1) * 512],
                            start=(hec == 0), stop=(hec == HEC - 1),
                        )
                    o_t = opool.tile([128, 512], F32, tag="o")
                    nc.vector.tensor_copy(o_t, ps)
                    nc.scalar.dma_start(
                        out=out_d[tb * 512 + t4 * 128 : tb * 512 + (t4 + 1) * 128,
                                  dh * 512 : (dh + 1) * 512],
                        in_=o_t,
                    )
    nc.finalize()
    return nc


_PROGRAM = None


def _get_program():
    global _PROGRAM
    if _PROGRAM is None:
        _PROGRAM = build_program()
    return _PROGRAM


def _shard_inputs(inputs):
    f = lambda a: np.ascontiguousarray(np.asarray(a, dtype=np.float32))
    x, wq, bq = f(inputs["x"]), f(inputs["Wq"]), f(inputs["bq"])
    wk, bk = f(inputs["Wk"]), f(inputs["bk"])
    wv, bv = f(inputs["Wv"]), f(inputs["bv"])
    wo = f(inputs["Wo"])
    in_maps = []
    for core in range(8):
        b, hh = core // 2, core % 2
        hs = slice(hh * 8, (hh + 1) * 8)
        in_maps.append({
            "x": x[b], "wq": wq[hs], "wk": wk[hs], "wv": wv[hs],
            "bq": bq[hs], "bk": bk[hs], "bv": bv[hs],
            "wo": wo[hh * 512 : (hh + 1) * 512],
        })
    return in_maps


def _install_ntff_shim():
    """bass_utils' axon trace path imports antenv.axon_hooks, which this image
    lacks. Provide it, backed by the ctypes NTFF driver in trn_agent_boot."""
    import sys as _sys
    import types

    if "antenv.axon_hooks" in _sys.modules:
        return
    try:
        if "/root/.axon_site" not in _sys.path:
            _sys.path.insert(0, "/root/.axon_site")
        from trn_agent_boot.trn_boot import _ntff_profile_via_ctypes

        hook = _ntff_profile_via_ctypes("/opt/axon/libaxon_pjrt.so")
    except Exception:
        hook = None
    mod = types.ModuleType("antenv.axon_hooks")
    mod.get_axon_ntff_profile_hook = lambda: hook
    mod.set_axon_ntff_profile_hook = lambda h: None
    _sys.modules["antenv.axon_hooks"] = mod


def kernel(**inputs):
    mask = int(np.asarray(inputs.get("mask", 1)))
    assert mask, "kernel is specialized for the causal (mask=1) case"
    bo = np.asarray(inputs["bo"], dtype=np.float32)

    from concourse.bass_utils import run_bass_kernel_spmd

    nc = _get_program()
    in_maps = _shard_inputs(inputs)
    trace = bool(int(os.environ.get("KERNEL_TRACE", "0")))
    if trace:
        _install_ntff_shim()
    res = run_bass_kernel_spmd(nc, in_maps, list(range(8)), trace=trace)
    outs = [np.asarray(r["out"], dtype=np.float32) for r in res.results]
    out = np.stack([outs[2 * b] + outs[2 * b + 1] for b in range(4)])
    out += bo
    kernel.last_exec_time_ns = res.exec_time_ns
    kernel.last_results = res
    return out


# revision 22
# speedup vs baseline: 3.6424x; 3.6424x over previous
"""Multi-head causal attention (B=4, T=2048, D=1024, H=16, hd=64) on 8 trn2 cores.

Sharding: core = (batch, head-half). Each core runs the full attention for its
batch element over 8 heads and produces the partial output projection against
its 512 rows of Wo. Host sums the two partials per batch and adds bo. No
device-to-device communication.

Per-core pipeline (Tile framework), v2:
  1. x -> xT via PE transpose instr (f32, 2 cycles/row); Q/K projections in
     f32r (full PE rate at K=128) writing fp16 qT/kT; V projected directly in
     [t, e] layout (stationary = xT chunk, moving = Wv for all 8 heads), so no
     separate V transpose.
  2. Per (head, 128-row q-block): scores = qT.T @ kT chunks in fp16 (1 c/r);
     causal masking + row-max of the diagonal 128-block via one
     tensor_mask_reduce (per-row exact mask, accum=max); shift C additionally
     samples the first <=256 columns. C bounds the true row max to within
     ~tail nats w.h.p.; margin -56 keeps exp finite and the denominator
     >= e^-56; exp + row-sum fused on ScalarE (accum_out).
  3. p (bf16) normalized by 1/sum on GpSimd (idle engine), transposed by the
     DMA xbar (sync engine issues).
  4. att @ V accumulated per 512-wide q-superblock (bf16 x bf16, fp32 PSUM).
  5. Output projection vs fp16 Wo shard; partial written to DRAM. x loads and
     out stores are issued from GpSimd to keep Scalar free for exp.
"""

import os
import sys
from contextlib import ExitStack

import numpy as np

if "/opt/trn_rl_repo" not in sys.path:
    sys.path.insert(0, "/opt/trn_rl_repo")

import concourse.bass as bass
import concourse.bacc as bacc
import concourse.mybir as mybir
import concourse.tile as tile
from concourse.masks import make_causal_mask, make_identity

F32 = mybir.dt.float32
F32R = mybir.dt.float32r
BF16 = mybir.dt.bfloat16
FP16 = mybir.dt.float16

MARGIN = -56.0


def r32(ap):
    return ap.bitcast(F32R)


USE_GPS_NORM = bool(int(os.environ.get("K_GPSN", "0")))
USE_XTR = bool(int(os.environ.get("K_XTR", "1")))
USE_FP16 = bool(int(os.environ.get("K_FP16", "1")))


def build_program(T=2048, D=1024, HL=8, HD=64):
    """One SPMD program: full attention for one batch element and HL local
    heads, plus the partial output projection (HL*HD rows of Wo).

    Fully fused pipeline over 512-row t-blocks: QKV(tb) -> shift C(tb) ->
    attention(qsb=tb, all heads) -> partial out-projection(tb). Keeps PE busy
    with projection matmuls while ScalarE runs exp for the previous block.
    """
    SCALE = float(HD) ** 0.5
    QKDT = FP16 if USE_FP16 else F32R
    FTDT = FP16 if USE_FP16 else BF16
    DC = D // 128          # contraction chunks for projections
    TB = T // 512          # 512-wide t blocks
    NSC = T // 128         # 128-wide s chunks
    HP = HL // 2           # head pairs
    HEC = (HL * HD) // 128 # feature contraction chunks for out proj
    DH = D // 512          # output column blocks

    nc = bacc.Bacc()

    x_d = nc.declare_dram_parameter("x", [T, D], F32, isOutput=False)
    wq_d = nc.declare_dram_parameter("wq", [HL, D, HD], F32R, isOutput=False)
    wk_d = nc.declare_dram_parameter("wk", [HL, D, HD], F32R, isOutput=False)
    wv_d = nc.declare_dram_parameter("wv", [HL, D, HD], F32R, isOutput=False)
    bq_d = nc.declare_dram_parameter("bq", [HL, HD], F32, isOutput=False)
    bk_d = nc.declare_dram_parameter("bk", [HL, HD], F32, isOutput=False)
    bv_d = nc.declare_dram_parameter("bv", [HL, HD], F32, isOutput=False)
    wo_d = nc.declare_dram_parameter("wo", [HL * HD, D], F32, isOutput=False)
    out_d = nc.declare_dram_parameter("out", [T, D], F32, isOutput=True)

    with tile.TileContext(nc) as tc, ExitStack() as ctx:
        const = ctx.enter_context(tc.tile_pool(name="const", bufs=1))
        pers = ctx.enter_context(tc.tile_pool(name="pers", bufs=1))
        small = ctx.enter_context(tc.tile_pool(name="small", bufs=8))
        xpool = ctx.enter_context(tc.tile_pool(name="xpool", bufs=2))
        xtpool = ctx.enter_context(tc.tile_pool(name="xtpool", bufs=1))
        qtpool = ctx.enter_context(tc.tile_pool(name="qtpool", bufs=2))
        ftpool = ctx.enter_context(tc.tile_pool(name="ftpool", bufs=2))
        ppool = ctx.enter_context(tc.tile_pool(name="ppool", bufs=3))
        ptpool = ctx.enter_context(tc.tile_pool(name="ptpool", bufs=2))
        opool = ctx.enter_context(tc.tile_pool(name="opool", bufs=2))
        ps512 = ctx.enter_context(tc.tile_pool(name="ps512", bufs=2, space="PSUM"))
        pssc = ctx.enter_context(tc.tile_pool(name="pssc", bufs=2, space="PSUM"))
        psv = ctx.enter_context(tc.tile_pool(name="psv", bufs=1, space="PSUM"))
        psO = ctx.enter_context(tc.tile_pool(name="psO", bufs=1, space="PSUM"))

        # ---- constants ----
        id_f32 = const.tile([128, 128], F32)
        make_identity(nc, id_f32)

        tri = const.tile([128, 128], F32)
        make_causal_mask(nc, tri, mask_val=-1.0e30)

        bqkv_sb = const.tile([128, 2, HP], F32)  # (h2 e) stacked per head pair
        for i, b_d in enumerate((bq_d, bk_d)):
            nc.scalar.dma_start(
                out=bqkv_sb[:, i, :],
                in_=b_d[:].rearrange("(hp h2) e -> (h2 e) hp", h2=2),
            )
        # bv broadcast to [t=128, (hp h2 e)=512] via PE outer product
        ones_row = const.tile([1, 128], F32)
        nc.gpsimd.memset(ones_row, 1.0)
        bv_row = const.tile([1, HL * HD], F32)
        nc.scalar.dma_start(out=bv_row, in_=bv_d[:].rearrange("g e -> (g e)"))
        bv_bc = const.tile([128, HL * HD], F32)
        ps_bc = psO.tile([128, 512], F32, tag="op", name="bvbc")
        nc.tensor.matmul(ps_bc, ones_row, bv_row, start=True, stop=True)
        nc.vector.tensor_copy(bv_bc, ps_bc)

        # prefetch the first two x chunks ahead of the weight stream
        x_pre = []
        for tc4 in range(2):
            x_tc = xpool.tile([128, D], F32, tag="x", name=f"x_pre{tc4}")
            nc.scalar.dma_start(out=x_tc, in_=x_d[tc4 * 128 : (tc4 + 1) * 128, :])
            x_pre.append(x_tc)
        # weights: one batched DMA per projection chunk
        w_sb = []
        for wi in range(3):
            wt = pers.tile([128, DC, HP, 2, HD], F32R, tag=f"w{wi}", name=f"w{wi}")
            w_sb.append(wt)
        for dc in range(DC):
            for wi, w_d in enumerate((wq_d, wk_d, wv_d)):
                nc.scalar.dma_start(
                    out=w_sb[wi][:, dc].rearrange("p hp h2 e -> p (hp h2) e"),
                    in_=w_d[:, dc * 128 : (dc + 1) * 128, :].rearrange(
                        "g d e -> d g e"
                    ),
                )
        # Wo: load f32 per hec chunk via the x-sized staging tag, cast to fp16
        wo_hf = pers.tile([128, HEC, D], FTDT)
        for hec in range(HEC):
            wo_st = xpool.tile([128, D], F32, tag="x", name=f"wo_st{hec}")
            nc.scalar.dma_start(
                out=wo_st, in_=wo_d[hec * 128 : (hec + 1) * 128, :]
            )
            nc.vector.tensor_copy(wo_hf[:, hec], wo_st)

        # ---- persistent activations (split per t block for precise deps) ----
        kT_t = [pers.tile([128, HP, 512], QKDT, tag=f"kT{i}", name=f"kT{i}") for i in range(TB)]
        v_t = [pers.tile([128, 4, HL * HD], BF16, tag=f"v{i}", name=f"v{i}") for i in range(TB)]

        for tb in range(TB):
            # ======== QKV projections for this t block ========
            xT_blk = xtpool.tile([128, DC, 512], F32R, tag="xT")
            qT_t = qtpool.tile([128, HP, 512], QKDT, tag="qT")
            for tc4 in range(4):
                if tb == 0 and tc4 < 2:
                    x_tc = x_pre[tc4]
                else:
                    x_tc = xpool.tile([128, D], F32, tag="x")
                    row0 = tb * 512 + tc4 * 128
                    nc.scalar.dma_start(out=x_tc, in_=x_d[row0 : row0 + 128, :])
                for dcg in range(DC // 4):
                    ps = ps512.tile([128, 4, 128], F32, tag="mm")
                    for j in range(4):
                        dc = dcg * 4 + j
                        if USE_XTR:
                            nc.tensor.transpose(
                                ps[:, j], x_tc[:, dc * 128 : (dc + 1) * 128],
                                id_f32,
                            )
                        else:
                            nc.tensor.matmul(
                                ps[:, j], x_tc[:, dc * 128 : (dc + 1) * 128],
                                id_f32, start=True, stop=True,
                            )
                    nc.vector.tensor_copy(
                        xT_blk[:, dcg * 4 : dcg * 4 + 4,
                               tc4 * 128 : (tc4 + 1) * 128],
                        ps,
                    )
            for hp in range(HP):
                for wi in range(2):
                    ps = ps512.tile([128, 512], F32, tag="mm")
                    for dc in range(DC):
                        nc.tensor.matmul(
                            ps, w_sb[wi][:, dc, hp], xT_blk[:, dc],
                            start=(dc == 0), stop=(dc == DC - 1),
                        )
                    bias_ap = bqkv_sb[:, wi, hp : hp + 1]
                    if wi == 0:
                        # fold the sqrt(hd) score scale into Q so the exp
                        # activation can run with scale=1.0 (keeps the
                        # -FLT_MAX causal-mask fill finite through exp)
                        nc.vector.tensor_scalar(
                            out=qT_t[:, hp], in0=ps,
                            scalar1=bias_ap, scalar2=SCALE,
                            op0=mybir.AluOpType.add,
                            op1=mybir.AluOpType.mult,
                        )
                    else:
                        nc.vector.tensor_scalar_add(kT_t[tb][:, hp], ps, bias_ap)
            # V directly in [t, (hp h2 e)] layout: stationary = xT chunk
            for tc4 in range(4):
                ps = ps512.tile([128, 512], F32, tag="mm")
                for dc in range(DC):
                    nc.tensor.matmul(
                        ps,
                        xT_blk[:, dc, tc4 * 128 : (tc4 + 1) * 128],
                        w_sb[2][:, dc],
                        start=(dc == 0), stop=(dc == DC - 1),
                    )
                nc.vector.tensor_add(v_t[tb][:, tc4], ps, bv_bc)

            # ======== attention for q superblock qsb == tb ========
            featT_t = ftpool.tile([128, HEC, 512], FTDT, tag="ft")
            smax = (tb + 1) * 4
            for hp in range(HP):
                pT_pair = [
                    ptpool.tile([128, NSC, 512], BF16, tag="pT", name=f"pT{i}")
                    for i in range(2)
                ]
                for q4 in range(4):
                    qb = tb * 4 + q4
                    slen = (qb + 1) * 128
                    nch = (slen + 1023) // 1024
                    p2 = [ppool.tile([128, T], BF16, tag="p", name=f"p{i}")
                          for i in range(2)]
                    sums2 = [small.tile([128, 2], F32, tag="sums",
                                        name=f"sums{i}") for i in range(2)]
                    cneg2 = [None, None]
                    for c in range(nch):
                        lo = c * 1024
                        w = min(1024, slen - lo)
                        ps2 = [pssc.tile([128, 1024], F32, tag="sc",
                                         name=f"sc{i}") for i in range(2)]
                        # paired row-disjoint matmuls on PE
                        for m0 in range(0, w, 512):
                            mw = min(512, w - m0)
                            kt = kT_t[(lo + m0) // 512]
                            for h2 in range(2):
                                e0, e1 = h2 * HD, (h2 + 1) * HD
                                nc.tensor.matmul(
                                    ps2[h2][:, m0 : m0 + mw],
                                    qT_t[e0:e1, hp,
                                         q4 * 128 : (q4 + 1) * 128],
                                    kt[e0:e1, hp, :mw],
                                    start=True, stop=True,
                                )
                        for h2 in range(2):
                            is_diag = c == nch - 1
                            if is_diag:
                                # causal mask of the diagonal 128 block
                                nc.vector.tensor_add(
                                    ps2[h2][:, w - 128 : w],
                                    ps2[h2][:, w - 128 : w], tri,
                                )
                            # shift C from cheap samples: the (masked) diag
                            # block max (nch==1) and/or the first <=256
                            # columns; cm <= true row max always, and trails
                            # it by few enough nats w.h.p. that margin -56
                            # keeps exp finite and the denominator >= e^-56
                            if c == 0 and qb >= 1:
                                kw = min(256, qb * 128)
                                cm_s = small.tile([128, 1], F32, tag="cm",
                                                  name=f"cms{h2}")
                                nc.vector.reduce_max(
                                    cm_s, ps2[h2][:, :kw],
                                    axis=mybir.AxisListType.X,
                                )
                            elif c == 0:
                                cm_s = None
                            if is_diag and nch == 1:
                                cm_d = small.tile([128, 1], F32, tag="cm",
                                                  name=f"cmd{h2}")
                                nc.vector.reduce_max(
                                    cm_d, ps2[h2][:, w - 128 : w],
                                    axis=mybir.AxisListType.X,
                                )
                            if c == 0:
                                if nch == 1 and cm_s is not None:
                                    cm = small.tile([128, 1], F32, tag="cm",
                                                    name=f"cmx{h2}")
                                    nc.vector.tensor_tensor(
                                        out=cm, in0=cm_d, in1=cm_s,
                                        op=mybir.AluOpType.max,
                                    )
                                elif nch == 1:
                                    cm = cm_d
                                else:
                                    cm = cm_s
                                cneg_t = small.tile([128, 1], F32, tag="cneg",
                                                    name=f"cneg{h2}")
                                nc.vector.tensor_scalar(
                                    out=cneg_t, in0=cm,
                                    scalar1=-1.0, scalar2=MARGIN,
                                    op0=mybir.AluOpType.mult,
                                    op1=mybir.AluOpType.add,
                                )
                                cneg2[h2] = cneg_t
                            nc.scalar.activation(
                                out=p2[h2][:, lo : lo + w], in_=ps2[h2][:, :w],
                                func=mybir.ActivationFunctionType.Exp,
                                bias=cneg2[h2], scale=1.0,
                                accum_out=sums2[h2][:, c : c + 1],
                            )
                    for h2 in range(2):
                        if nch == 1:
                            ssum = sums2[h2][:, 0:1]
                        else:
                            ssum = small.tile([128, 1], F32, tag="ssum")
                            nc.vector.tensor_tensor(
                                out=ssum, in0=sums2[h2][:, 0:1],
                                in1=sums2[h2][:, 1:2], op=mybir.AluOpType.add,
                            )
                        rinv = small.tile([128, 1], F32, tag="rinv")
                        nc.vector.reciprocal(rinv, ssum)
                        norm_eng = nc.gpsimd if USE_GPS_NORM else nc.vector
                        norm_eng.tensor_scalar_mul(
                            p2[h2][:, :slen], p2[h2][:, :slen], rinv
                        )
                        nc.sync.dma_start_transpose(
                            out=pT_pair[h2][:, 0 : qb + 1,
                                            q4 * 128 : (q4 + 1) * 128],
                            in_=p2[h2][:, :slen],
                        )
                # att @ V: both heads interleaved -> col-groups 0-63 / 64-127
                psva = psv.tile([128, 512], F32, tag="attv")
                for sc in range(smax):
                    j = max(0, sc - tb * 4)
                    t_lo = j * 128
                    for h2 in range(2):
                        nc.tensor.matmul(
                            psva[h2 * HD : (h2 + 1) * HD, t_lo:],
                            v_t[sc // 4][:, sc % 4,
                                         hp * 128 + h2 * HD :
                                         hp * 128 + h2 * HD + HD],
                            pT_pair[h2][:, sc, t_lo:],
                            start=(sc == 0), stop=(sc == smax - 1),
                            skip_group_check=True,
                        )
                nc.vector.tensor_copy(featT_t[:, hp], psva)

            # ======== partial out-projection for this t block ========
            ftv = featT_t.rearrange("p hec (t4 t) -> p hec t4 t", t=128)
            for t4 in range(4):
                for dh in range(DH):
                    ps = psO.tile([128, 512], F32, tag="op")
                    for hec in range(HEC):
                        nc.tensor.matmul(
                            ps, ftv[:, hec, t4],
                            wo_hf[:, hec, dh * 512 : (dh + 1) * 512],
                            start=(hec == 0), stop=(hec == HEC - 1),
                        )
                    o_t = opool.tile([128, 512], F32, tag="o")
                    nc.vector.tensor_copy(o_t, ps)
                    nc.scalar.dma_start(
                        out=out_d[tb * 512 + t4 * 128 : tb * 512 + (t4 + 1) * 128,
                                  dh * 512 : (dh + 1) * 512],
                        in_=o_t,
                    )
    nc.finalize()
    return nc


_PROGRAM = None


def _get_program():
    global _PROGRAM
    if _PROGRAM is None:
        _PROGRAM = build_program()
    return _PROGRAM


def _shard_inputs(inputs):
    f = lambda a: np.ascontiguousarray(np.asarray(a, dtype=np.float32))
    x, wq, bq = f(inputs["x"]), f(inputs["Wq"]), f(inputs["bq"])
    wk, bk = f(inputs["Wk"]), f(inputs["bk"])
    wv, bv = f(inputs["Wv"]), f(inputs["bv"])
    wo = f(inputs["Wo"])
    in_maps = []
    for core in range(8):
        b, hh = core // 2, core % 2
        hs = slice(hh * 8, (hh + 1) * 8)
        in_maps.append({
            "x": x[b], "wq": wq[hs], "wk": wk[hs], "wv": wv[hs],
            "bq": bq[hs], "bk": bk[hs], "bv": bv[hs],
            "wo": wo[hh * 512 : (hh + 1) * 512],
        })
    return in_maps


def _install_ntff_shim():
    """bass_utils' axon trace path imports antenv.axon_hooks, which this image
    lacks. Provide it, backed by the ctypes NTFF driver in trn_agent_boot."""
    import sys as _sys
    import types

    if "antenv.axon_hooks" in _sys.modules:
        return
    try:
        if "/root/.axon_site" not in _sys.path:
            _sys.path.insert(0, "/root/.axon_site")
        from trn_agent_boot.trn_boot import _ntff_profile_via_ctypes

        hook = _ntff_profile_via_ctypes("/opt/axon/libaxon_pjrt.so")
    except Exception:
        hook = None
    mod = types.ModuleType("antenv.axon_hooks")
    mod.get_axon_ntff_profile_hook = lambda: hook
    mod.set_axon_ntff_profile_hook = lambda h: None
    _sys.modules["antenv.axon_hooks"] = mod


def kernel(**inputs):
    mask = int(np.asarray(inputs.get("mask", 1)))
    assert mask, "kernel is specialized for the causal (mask=1) case"
    bo = np.asarray(inputs["bo"], dtype=np.float32)

    from concourse.bass_utils import run_bass_kernel_spmd

    nc = _get_program()
    in_maps = _shard_inputs(inputs)
    trace = bool(int(os.environ.get("KERNEL_TRACE", "0")))
    if trace:
        _install_ntff_shim()
    res = run_bass_kernel_spmd(nc, in_maps, list(range(8)), trace=trace)
    outs = [np.asarray(r["out"], dtype=np.float32) for r in res.results]
    out = np.stack([outs[2 * b] + outs[2 * b + 1] for b in range(4)])
    out += bo
    kernel.last_exec_time_ns = res.exec_time_ns
    kernel.last_results = res
    return out
